# revision 14
# baseline (speedup 1.0000x reference)
"""Trainium2 Bass kernel for a ViT-style transformer block (pre-norm).

Strategy:
  - Pure data parallelism: 64 batches -> 8 per NeuronCore, no collectives.
  - Activations kept feature-major on device (xT: [D, tokens]) so every
    matmul contracts over the partition dimension with no transposes;
    the host transposes x on the way in and the output on the way out.
  - LayerNorm affine params and all biases are folded on the host into the
    adjacent weight matrices / bias vectors, so the device only computes
    the normalize step (x * alpha + beta with per-token alpha/beta).
  - All matmul operands are bf16 (weights converted on host, activations
    rounded on the PSUM->SBUF copy); accumulation stays fp32 in PSUM.
  - LN mean / sum-of-squares via ones-matrix matmuls whose lhsT is
    [128, 128] of ones, so the per-token stats come out of the PE already
    broadcast across all 128 partitions (no gpsimd broadcast needed).
  - Attention per (batch, head) in scoresT layout ([key, query]); the
    softmax denominator rides along as a 65th column of V (the mask
    vector), so no separate denominator matmuls.
  - Scalar engine only ever runs Exp in the attention phase and Gelu in
    the FFN phase; the LN rsqrt calls are batched (LN1 up front, LN2 at
    the start of the FFN phase) to avoid activation-table thrash.
  - FFN streams W1 tiles from HBM; W2 is loaded into the same SBUF buffer
    that held Wq/Wk/Wv/Wo during attention.
"""

import numpy as np
import ml_dtypes

import concourse.bacc as bacc
import concourse.mybir as mybir
from concourse.bass_utils import run_bass_kernel_spmd
from concourse.tile import TileContext

F32 = mybir.dt.float32
F32R = mybir.dt.float32r
BF16 = mybir.dt.bfloat16
AF = mybir.ActivationFunctionType
OP = mybir.AluOpType

N_CORES = 8
B, S, D, H, FF = 64, 197, 768, 12, 3072
DH = D // H  # 64
EPS = 1e-6
P = 128
CT = D // P  # 6 contraction tiles
FT = FF // P  # 24
GB = 2  # batches per group
DV = DH + 1  # V columns per head incl. denominator column


def build_nc(n_cores=N_CORES, b_shard=8):
    """Build + compile the per-core kernel. b_shard = batches per core."""
    NG = b_shard // GB  # groups (= FFN chunks)
    T = b_shard * S  # tokens per core
    GT = GB * S  # tokens per group (394)

    nc = bacc.Bacc(
        "TRN2", target_bir_lowering=False, debug=False, num_devices=n_cores
    )

    xt_d = nc.dram_tensor("xt", [D, T], F32, kind="ExternalInput")
    wq_d = nc.dram_tensor("wq", [D, D], BF16, kind="ExternalInput")
    wk_d = nc.dram_tensor("wk", [D, D], BF16, kind="ExternalInput")
    wv_d = nc.dram_tensor("wv", [D, D], BF16, kind="ExternalInput")
    wo_d = nc.dram_tensor("wo", [D, D], BF16, kind="ExternalInput")
    w1_d = nc.dram_tensor("w1", [D, FF], BF16, kind="ExternalInput")
    w2_d = nc.dram_tensor("w2", [FF, D], BF16, kind="ExternalInput")
    bq_d = nc.dram_tensor("bq", [D], F32, kind="ExternalInput")
    bk_d = nc.dram_tensor("bk", [D], F32, kind="ExternalInput")
    bo_d = nc.dram_tensor("bo", [D], F32, kind="ExternalInput")
    b1_d = nc.dram_tensor("b1", [FF], F32, kind="ExternalInput")
    b2_d = nc.dram_tensor("b2", [D], F32, kind="ExternalInput")
    mk_d = nc.dram_tensor("mk", [P, 2 * b_shard], F32, kind="ExternalInput")
    yt_d = nc.dram_tensor("yt", [D, T], F32, kind="ExternalOutput")

    def pon(ap_1d):  # [ (o p) ] -> [p, o]
        return ap_1d.rearrange("(o p) -> p o", p=P)

    def ponn(ap_2d):  # [(o p), n] -> [p, o, n]
        return ap_2d.rearrange("(o p) n -> p o n", p=P)

    with TileContext(nc) as tc:
        with (
            tc.tile_pool(name="const", bufs=1) as const,
            tc.tile_pool(name="xres", bufs=1) as xres,
            tc.tile_pool(name="sml", bufs=1) as sml,
            tc.tile_pool(name="sqp", bufs=2) as sqpool,
        ):
            # Resident weight buffer: Wq/Wk/Wv/Wo during attention, W2 later.
            wbuf = const.tile([P, 4 * CT, D], BF16, tag="wbuf", name="wbuf")

            bq_sb = const.tile([P, CT], F32, tag="bq", name="bq_sb")
            bk_sb = const.tile([P, CT], F32, tag="bk", name="bk_sb")
            bo_sb = const.tile([P, CT], F32, tag="bo", name="bo_sb")
            b2_sb = const.tile([P, CT], F32, tag="b2", name="b2_sb")
            b1_sb = const.tile([P, FT], F32, tag="b1", name="b1_sb")
            mk_sb = const.tile([P, 2 * b_shard], F32, tag="mk", name="mk_sb")
            ones = const.tile([P, 1], F32, tag="ones", name="ones_sb")
            onesb = const.tile([P, P], BF16, tag="onesb", name="onesb_sb")
            eps_sb = const.tile([P, 1], F32, tag="eps", name="eps_sb")
            nc.vector.memset(eps_sb[:], EPS)
            nc.sync.dma_start(out=bq_sb[:], in_=pon(bq_d[:]))
            nc.sync.dma_start(out=bk_sb[:], in_=pon(bk_d[:]))
            nc.sync.dma_start(out=bo_sb[:], in_=pon(bo_d[:]))
            nc.sync.dma_start(out=b2_sb[:], in_=pon(b2_d[:]))
            nc.sync.dma_start(out=b1_sb[:], in_=pon(b1_d[:]))
            nc.sync.dma_start(out=mk_sb[:], in_=mk_d[:])
            nc.vector.memset(ones[:], 1.0)
            nc.vector.tensor_scalar_mul(
                onesb[:], ones[:, 0:1].to_broadcast((P, P)), 1.0
            )

            # Residual stream, one tile per group/chunk. The first chunk and
            # the q/k weights land first so LN1/QKV of group 0 start early.
            xt_g = []
            for g in range(NG):
                xg = xres.tile([P, CT, GT], F32, tag=f"xt{g}", name=f"xt{g}")
                nc.sync.dma_start(
                    out=xg[:], in_=ponn(xt_d[:])[:, :, g * GT : (g + 1) * GT]
                )
                xt_g.append(xg)
                if g == 0:
                    nc.sync.dma_start(out=wbuf[:, 0:CT, :], in_=ponn(wq_d[:]))
                    nc.sync.dma_start(
                        out=wbuf[:, CT : 2 * CT, :], in_=ponn(wk_d[:])
                    )
                    nc.sync.dma_start(
                        out=wbuf[:, 2 * CT : 3 * CT, :], in_=ponn(wv_d[:])
                    )
                if g == min(1, NG - 1):
                    nc.sync.dma_start(
                        out=wbuf[:, 3 * CT : 4 * CT, :], in_=ponn(wo_d[:])
                    )

            def ln_sums(psS, xg, tag_pfx):
                """Matmul per-token sum / sum-of-squares of one group.

                lhsT is a [128, 128] ones matrix, so the [1, N] stats come
                out of the PE replicated across all 128 partitions.
                """
                ps_sum = psS.tile([P, GT], F32, tag="mm", bufs=2, name=f"{tag_pfx}_su")
                ps_sq = psS.tile([P, GT], F32, tag="mm", bufs=2, name=f"{tag_pfx}_sq")
                for ct in range(CT):
                    xr = sqpool.tile([P, GT], BF16, tag="xr", name=f"{tag_pfx}_x{ct}")
                    sq = sqpool.tile([P, GT], BF16, tag="sq", name=f"{tag_pfx}_s{ct}")
                    nc.vector.tensor_scalar_mul(xr[:], xg[:, ct, :], 1.0)
                    nc.vector.tensor_mul(sq[:], xr[:], xr[:])
                    nc.tensor.matmul(
                        ps_sum[:], onesb[:], xr[:],
                        start=(ct == 0), stop=(ct == CT - 1),
                    )
                    nc.tensor.matmul(
                        ps_sq[:], onesb[:], sq[:],
                        start=(ct == 0), stop=(ct == CT - 1),
                    )
                return ps_sum, ps_sq

            def ln_tail(ps_sum, ps_sq, alpha, beta, scratch):
                """alpha = rsqrt(var+eps), beta = -mean*alpha; all [128, N]."""
                nc.vector.tensor_scalar_mul(scratch[:], ps_sum[:], 1.0 / D)
                nc.vector.tensor_mul(alpha[:], scratch[:], scratch[:])
                nc.vector.scalar_tensor_tensor(
                    alpha[:], ps_sq[:], 1.0 / D, alpha[:],
                    op0=OP.mult, op1=OP.subtract,
                )
                nc.scalar.activation(
                    beta[:], alpha[:], AF.Sqrt, bias=eps_sb[:, 0:1], scale=1.0
                )
                nc.vector.reciprocal_approx_fast(out=alpha[:], in_=beta[:])
                nc.vector.scalar_tensor_tensor(
                    beta[:], scratch[:], -1.0, alpha[:], op0=OP.mult, op1=OP.mult
                )

            def ln_apply(xh, xg, alpha, beta, tag_pfx):
                """xh (bf16) = xg * alpha + beta, one fp32 intermediate."""
                for ct in range(CT):
                    tmp = sqpool.tile(
                        [P, GT], F32, tag="tmp", name=f"{tag_pfx}_t{ct}"
                    )
                    nc.vector.tensor_mul(tmp[:], xg[:, ct, :], alpha[:])
                    nc.vector.tensor_add(xh[:, ct, :], tmp[:], beta[:])

            ln2_sums = []
            # ---------------- Phase A: attention ----------------
            with (
                tc.tile_pool(name="psA", bufs=1, space="PSUM") as psA,
                tc.tile_pool(name="psB", bufs=1, space="PSUM") as psB,
                tc.tile_pool(name="psC", bufs=1, space="PSUM") as psC,
                tc.tile_pool(name="attw", bufs=2) as attw,
                tc.tile_pool(name="attx", bufs=3) as attx,
            ):
                # LN1 stats for all groups up front: the only Scalar-engine
                # table in the rest of phase A is Exp.
                ln1_ab = []
                for g in range(NG):
                    ps_sum, ps_sq = ln_sums(psA, xt_g[g], f"ln1g{g}")
                    alpha = sml.tile([P, GT], F32, tag=f"a1_{g}", name=f"a1_{g}")
                    beta = sml.tile([P, GT], F32, tag=f"b1_{g}", name=f"b1_{g}")
                    scr = sqpool.tile([P, GT], F32, tag="tmp", name=f"m1_{g}")
                    ln_tail(ps_sum, ps_sq, alpha, beta, scr)
                    ln1_ab.append((alpha, beta))

                def prep(g):
                    # LN1 apply + Q/K/V projections for group g.
                    xg = xt_g[g]
                    alpha, beta = ln1_ab[g]
                    xh = attw.tile([P, CT, GT], BF16, tag="xh", name=f"xh{g}")
                    ln_apply(xh, xg, alpha, beta, f"l1a{g}")

                    qT = attw.tile([P, CT, GB, S], BF16, tag="qT", name=f"qT{g}")
                    kT = attw.tile([P, CT, GB, S], BF16, tag="kT", name=f"kT{g}")
                    for dst, wofs, bias in ((qT, 0, bq_sb), (kT, CT, bk_sb)):
                        for mt in range(CT):
                            ps = psA.tile(
                                [P, GT], F32, tag="mm", bufs=2,
                                name=f"psqk{g}_{wofs}_{mt}",
                            )
                            for ct in range(CT):
                                nc.tensor.matmul(
                                    ps[:],
                                    wbuf[:, wofs + ct, mt * P : (mt + 1) * P],
                                    xh[:, ct, :],
                                    start=(ct == 0), stop=(ct == CT - 1),
                                )
                            nc.vector.tensor_scalar_add(
                                dst[:, mt, :, :],
                                ps[:].rearrange("p (b s) -> p b s", b=GB),
                                bias[:, mt : mt + 1],
                            )

                    # V in token-major layout, rows scaled by the attention
                    # mask; column DH holds the raw mask value so the AV
                    # matmul also produces the softmax denominator.
                    vT = attw.tile(
                        [P, GB, 2, H, DV], BF16, tag="vT", name=f"vT{g}"
                    )
                    for b2 in range(GB):
                        for tt in range(2):
                            off = b2 * S + tt * P
                            M = P if tt == 0 else S - P
                            mi = (g * GB + b2) * 2 + tt
                            for hf in range(2):
                                ps = psA.tile(
                                    [P, D // 2], F32, tag="mm", bufs=2,
                                    name=f"psv{g}_{b2}_{tt}_{hf}",
                                )
                                for ct in range(CT):
                                    nc.tensor.matmul(
                                        ps[:M, :],
                                        xh[:, ct, off : off + M],
                                        wbuf[
                                            :, 2 * CT + ct,
                                            hf * (D // 2) : (hf + 1) * (D // 2),
                                        ],
                                        start=(ct == 0), stop=(ct == CT - 1),
                                    )
                                nc.vector.tensor_scalar_mul(
                                    vT[
                                        0:M, b2, tt,
                                        hf * (H // 2) : (hf + 1) * (H // 2),
                                        0:DH,
                                    ],
                                    ps[:M, :].rearrange("p (h d) -> p h d", h=H // 2),
                                    mk_sb[0:M, mi : mi + 1],
                                )
                            nc.vector.tensor_scalar_mul(
                                vT[0:M, b2, tt, :, DH:DV],
                                ones[0:M, 0:1].to_broadcast((M, H, 1)),
                                mk_sb[0:M, mi : mi + 1],
                            )
                    return qT, kT, vT

                preps = {0: prep(0)}
                for g in range(NG):
                    qT, kT, vT = preps.pop(g)
                    xg = xt_g[g]

                    attnT = attw.tile([P, CT, GT], BF16, tag="attnT", name=f"at{g}")
                    for b2 in range(GB):
                        for h in range(H):
                            hp, rh = h // 2, (h % 2) * DH
                            ps_sc = psB.tile(
                                [P, 2, S], F32, tag="sc", bufs=3,
                                name=f"s_{g}{b2}{h}",
                            )
                            nc.tensor.matmul(
                                ps_sc[:, 0, :],
                                kT[rh : rh + DH, hp, b2, 0:P],
                                qT[rh : rh + DH, hp, b2, :],
                                start=True, stop=True,
                            )
                            nc.tensor.matmul(
                                ps_sc[0 : S - P, 1, :],
                                kT[rh : rh + DH, hp, b2, P:S],
                                qT[rh : rh + DH, hp, b2, :],
                                start=True, stop=True,
                            )
                            expT = attx.tile(
                                [P, 2, S], BF16, tag="exp", name=f"e_{g}{b2}{h}"
                            )
                            nc.scalar.activation(
                                expT[:, 0, :], ps_sc[:, 0, :], AF.Exp, scale=1.0
                            )
                            nc.scalar.activation(
                                expT[0 : S - P, 1, :], ps_sc[0 : S - P, 1, :],
                                AF.Exp, scale=1.0,
                            )
                            # AV matmul; row DH of the output is the softmax
                            # denominator (V's mask column).
                            ps_a = psC.tile(
                                [DV, S], F32, tag="at", bufs=3, name=f"a_{g}{b2}{h}"
                            )
                            nc.tensor.matmul(
                                ps_a[:, :],
                                vT[:, b2, 0, h, :],
                                expT[:, 0, :],
                                start=True, stop=False,
                            )
                            nc.tensor.matmul(
                                ps_a[:, :],
                                vT[0 : S - P, b2, 1, h, :],
                                expT[0 : S - P, 1, :],
                                start=False, stop=True,
                            )
                            # The custom-DVE reciprocal is partition-locked;
                            # copy the denominator row down to partition 0
                            # with a plain DVE op (32-aligned shifts are ok).
                            den = attx.tile(
                                [1, S], F32, tag="den", name=f"dn_{g}{b2}{h}"
                            )
                            nc.vector.tensor_scalar_mul(
                                den[0:1, :], ps_a[DH : DH + 1, :], 1.0
                            )
                            r_rep = attx.tile(
                                [DH, S], F32, tag="rrep", name=f"rr_{g}{b2}{h}"
                            )
                            nc.vector.reciprocal_approx_fast(
                                out=r_rep[0:1, :], in_=den[0:1, :]
                            )
                            nc.gpsimd.partition_broadcast(
                                r_rep[:], r_rep[0:1, :]
                            )
                            nc.vector.tensor_mul(
                                attnT[rh : rh + DH, hp, b2 * S : (b2 + 1) * S],
                                ps_a[0:DH, :],
                                r_rep[:],
                            )

                    # Prep the next group before the output projection so
                    # the tensor queue has work while the last heads' softmax
                    # tails (DMA bounce + reciprocal + broadcast) drain.
                    if g + 1 < NG:
                        preps[g + 1] = prep(g + 1)

                    # Output projection + residual (in place into xg).
                    for mt in range(CT):
                        ps = psA.tile(
                            [P, GT], F32, tag="mm", bufs=2, name=f"pso{g}_{mt}"
                        )
                        for ct in range(CT):
                            nc.tensor.matmul(
                                ps[:],
                                wbuf[:, 3 * CT + ct, mt * P : (mt + 1) * P],
                                attnT[:, ct, :],
                                start=(ct == 0), stop=(ct == CT - 1),
                            )
                        nc.vector.scalar_tensor_tensor(
                            xg[:, mt, :], ps[:], bo_sb[:, mt : mt + 1],
                            xg[:, mt, :], op0=OP.add, op1=OP.add,
                        )

                    # LN2 raw sums for this chunk, overlapped with attention
                    # of the remaining groups; reduced to SBUF so the PSUM
                    # bank frees up (rsqrt tails run at the start of phase B).
                    ps_sum, ps_sq = ln_sums(psA, xg, f"ln2c{g}")
                    sm = sml.tile([P, GT], F32, tag=f"s2_{g}", name=f"s2_{g}")
                    sq = sml.tile([P, GT], F32, tag=f"q2_{g}", name=f"q2_{g}")
                    nc.vector.tensor_scalar_mul(sm[:], ps_sum[:], 1.0)
                    nc.vector.tensor_scalar_mul(sq[:], ps_sq[:], 1.0)
                    ln2_sums.append((sm, sq))

            # ---------------- Phase B: FFN ----------------
            with (
                tc.tile_pool(name="psU", bufs=1, space="PSUM") as psU,
                tc.tile_pool(name="psY", bufs=1, space="PSUM") as psY,
                tc.tile_pool(name="ffw", bufs=1) as ffw,
            ):
                # W2 replaces Wq..Wo in the resident weight buffer; split the
                # load so the first FFN chunk can start before the tail lands.
                for wc in range(4):
                    nc.sync.dma_start(
                        out=wbuf[:, wc * CT : (wc + 1) * CT, :],
                        in_=ponn(w2_d[:])[:, wc * CT : (wc + 1) * CT, :],
                    )
                # LN2 tails for all chunks: one Rsqrt table load, then the
                # Scalar engine only runs Gelu.
                ln2_ab = []
                for c in range(NG):
                    sm, sq = ln2_sums[c]
                    alpha = sml.tile([P, GT], F32, tag=f"s2_{c}", name=f"a2_{c}")
                    beta = sml.tile([P, GT], F32, tag=f"q2_{c}", name=f"b2_{c}")
                    scr = sqpool.tile([P, GT], F32, tag="tmp", name=f"m2_{c}")
                    ln_tail(sm, sq, alpha, beta, scr)
                    ln2_ab.append((alpha, beta))
                for c in range(NG):
                    xg = xt_g[c]
                    alpha, beta = ln2_ab[c]
                    xh = ffw.tile([P, CT, GT], BF16, tag="xh2", bufs=2, name=f"xh2_{c}")
                    ln_apply(xh, xg, alpha, beta, f"l2a{c}")

                    ps_y = [
                        psY.tile([P, GT], F32, tag=f"y{mt}", name=f"psy{c}_{mt}")
                        for mt in range(CT)
                    ]
                    for ft in range(FT):
                        w1t = ffw.tile(
                            [P, CT, P], BF16, tag="w1", bufs=3, name=f"w1_{c}_{ft}"
                        )
                        nc.sync.dma_start(
                            out=w1t[:], in_=ponn(w1_d[:])[:, :, ft * P : (ft + 1) * P]
                        )
                        ps_u = psU.tile(
                            [P, GT], F32, tag="st_sum", bufs=2, name=f"psu{c}_{ft}"
                        )
                        for ct in range(CT):
                            nc.tensor.matmul(
                                ps_u[:],
                                w1t[:, ct, :],
                                xh[:, ct, :],
                                start=(ct == 0), stop=(ct == CT - 1),
                            )
                        g_sb = ffw.tile([P, GT], BF16, tag="g", bufs=3, name=f"g{c}_{ft}")
                        nc.scalar.activation(
                            g_sb[:], ps_u[:], AF.Gelu,
                            bias=b1_sb[:, ft : ft + 1], scale=1.0,
                        )
                        for mt in range(CT):
                            nc.tensor.matmul(
                                ps_y[mt][:],
                                wbuf[:, ft, mt * P : (mt + 1) * P],
                                g_sb[:],
                                start=(ft == 0), stop=(ft == FT - 1),
                            )
                    for mt in range(CT):
                        nc.vector.scalar_tensor_tensor(
                            xg[:, mt, :], ps_y[mt][:], b2_sb[:, mt : mt + 1],
                            xg[:, mt, :], op0=OP.add, op1=OP.add,
                        )
                    nc.sync.dma_start(
                        out=ponn(yt_d[:])[:, :, c * GT : (c + 1) * GT], in_=xg[:]
                    )

    nc.compile()
    return nc


def to_bf16(a):
    return np.ascontiguousarray(a, np.float32).astype(ml_dtypes.bfloat16)


def host_prep(inputs, b_shard=8):
    """Fold LN affine + biases into weights; build per-core input maps."""
    f = np.float32
    x = np.ascontiguousarray(inputs["x"], dtype=f)
    Wq, bq = np.asarray(inputs["Wq"], f), np.asarray(inputs["bq"], f)
    Wk, bk = np.asarray(inputs["Wk"], f), np.asarray(inputs["bk"], f)
    Wv, bv = np.asarray(inputs["Wv"], f), np.asarray(inputs["bv"], f)
    Wo, bo = np.asarray(inputs["Wo"], f), np.asarray(inputs["bo"], f)
    W1, b1 = np.asarray(inputs["W1"], f), np.asarray(inputs["b1"], f)
    W2, b2 = np.asarray(inputs["W2"], f), np.asarray(inputs["b2"], f)
    ln1w, ln1b = np.asarray(inputs["ln1_w"], f), np.asarray(inputs["ln1_b"], f)
    ln2w, ln2b = np.asarray(inputs["ln2_w"], f), np.asarray(inputs["ln2_b"], f)
    mask = np.asarray(inputs["mask"])

    s = f(1.0 / np.sqrt(DH))
    wq_e = np.ascontiguousarray((ln1w[:, None] * Wq) * s)
    bq_e = (ln1b @ Wq + bq) * s
    wk_e = np.ascontiguousarray(ln1w[:, None] * Wk)
    bk_e = ln1b @ Wk + bk
    wv_e = np.ascontiguousarray(ln1w[:, None] * Wv)
    bv_e = ln1b @ Wv + bv
    bo_e = bv_e @ Wo + bo
    w1_e = np.ascontiguousarray(ln2w[:, None] * W1)
    b1_e = ln2b @ W1 + b1

    mask_f = mask.astype(f)  # [B, S]

    wq_b, wk_b, wv_b, wo_b = to_bf16(wq_e), to_bf16(wk_e), to_bf16(wv_e), to_bf16(Wo)
    w1_b, w2_b = to_bf16(w1_e), to_bf16(W2)

    n_cores = B // b_shard
    in_maps = []
    for c in range(n_cores):
        xs = x[c * b_shard : (c + 1) * b_shard]  # [b_shard, S, D]
        xt = np.ascontiguousarray(
            xs.transpose(2, 0, 1).reshape(D, b_shard * S)
        )
        mk = np.zeros((P, 2 * b_shard), f)
        ms = mask_f[c * b_shard : (c + 1) * b_shard]  # [b_shard, S]
        for b_ in range(b_shard):
            mk[:, 2 * b_] = ms[b_, 0:P]
            mk[0 : S - P, 2 * b_ + 1] = ms[b_, P:S]
        in_maps.append(
            {
                "xt": xt,
                "wq": wq_b, "wk": wk_b, "wv": wv_b, "wo": wo_b,
                "w1": w1_b, "w2": w2_b,
                "bq": bq_e, "bk": bk_e, "bo": bo_e,
                "b1": b1_e, "b2": b2, "mk": mk,
            }
        )
    return in_maps


_NC_CACHE = {}


def get_nc(n_cores=N_CORES, b_shard=8):
    key = (n_cores, b_shard)
    if key not in _NC_CACHE:
        _NC_CACHE[key] = build_nc(n_cores, b_shard)
    return _NC_CACHE[key]


def kernel(**inputs):
    b_shard = B // N_CORES
    nc = get_nc(N_CORES, b_shard)
    in_maps = host_prep(inputs, b_shard)
    res = run_bass_kernel_spmd(nc, in_maps, list(range(N_CORES)))
    outs = []
    for c in range(N_CORES):
        yt = res.results[c]["yt"]  # [D, b_shard*S]
        outs.append(yt.reshape(D, b_shard, S).transpose(1, 2, 0))
    return np.ascontiguousarray(np.concatenate(outs, axis=0), dtype=np.float32)
